# revision 1
# baseline (speedup 1.0000x reference)
"""Trainium2 Bass kernel for nn_BertWordPair (ragged RoPE pair scores).

Strategy
--------
Inputs: qw, kw (B=8, S=768, H=4, D=256) fp32; token_index, thread_id (S,) int32.
Output: (B, S, S, H) fp32 where each (row-block, col-block) pair of the 6x128
thread-block grid uses one of three RoPE sign regimes:
    pp: rope(q,+pos) . rope(k,+pos)
    np: rope(q,-pos) . rope(k,+pos)   (0 < ti_r < ti_c)
    pn: rope(q,+pos) . rope(k,-pos)   (ti_c > 0, ti_r > ti_c)

Host side precomputes the rotated variants q+, q-, k+ in a de-interleaved
(pair-index, token) layout, casts to fp16, and shards batch across the 8
cores (1 dialogue per core). k- is derived on-device from k+ by a DVE
fp16 rotation (k- = R(-2θ)k+, small cos2θ/sin2θ table) to save its DMA.
Device work: matmuls (one 128x128x256 contraction per output block/head,
fp16 in, fp32 PSUM, 4 heads packed per PSUM bank), one head-interleaving
PSUM->SBUF copy per bank (ACT early rows, DVE/ACT later), and half-row
output DMAs. The DMA ring is ordered so the timeline is gapless:
~2.0us Tile preamble + ~39.0us DMA (14.0MB @ ~360GB/s, zero idle) +
~1.6us tail = ~42.6us per core (cost-model).
"""

import os

import numpy as np

ROPE_BASE = 10000.0
B, S, H, D = 8, 768, 4, 256
HALF = D // 2  # 128
BLK = 128
NB = S // BLK  # 6
N_CORES = 8

_prog_cache = {}


def _host_rotations(qw, kw, token_index):
    """Return u/v (even/odd) rotated variants, fp32.

    Shapes: (B, S, H, HALF) each for (qp_u, qp_v, qn_u, qn_v, kp_u, kp_v,
    kn_u, kn_v)."""
    inv_freq = np.power(
        np.float32(ROPE_BASE),
        (np.arange(HALF, dtype=np.float32) * np.float32(-2.0 / D)),
    )  # (HALF,)
    pos = token_index.astype(np.float32)  # (S,)
    theta = pos[:, None] * inv_freq[None, :]  # (S, HALF)
    cos = np.cos(theta)[None, :, None, :]  # (1,S,1,HALF)
    sin = np.sin(theta)[None, :, None, :]

    out = []
    for x in (qw, kw):
        u = x[..., 0::2]  # (B,S,H,HALF)
        v = x[..., 1::2]
        uc = u * cos
        vs = v * sin
        vc = v * cos
        us = u * sin
        # positive rotation
        out.append((uc - vs, vc + us))
        # negative rotation (sin -> -sin)
        out.append((uc + vs, vc - us))
    return out  # [(qp_u,qp_v),(qn_u,qn_v),(kp_u,kp_v),(kn_u,kn_v)]


def _to_device_layout(u, v, blocks):
    """(B,S,H,HALF) u/v -> (B, H, 2, HALF, T) fp16 for the given token blocks."""
    cols = np.concatenate([np.arange(b * BLK, (b + 1) * BLK) for b in blocks])
    u = u[:, cols]  # (B,T,H,HALF)
    v = v[:, cols]
    arr = np.stack([u, v], axis=2)  # (B,T,2,H,HALF)
    arr = np.transpose(arr, (0, 3, 2, 4, 1))  # (B,H,2,HALF,T)
    return np.ascontiguousarray(arr.astype(np.float16))


def _regime_map(thread_id):
    """Return (regimes, ok). regimes[i][j] in {'pp','np','pn'} per 128-block."""
    tid = np.asarray(thread_id)
    if tid.shape[0] != S:
        return None, False
    blocks = tid.reshape(NB, BLK)
    if not np.all(blocks == blocks[:, :1]):
        return None, False  # thread blocks not aligned to 128 grid
    tvals = blocks[:, 0]
    regimes = []
    for i in range(NB):
        row = []
        for j in range(NB):
            ti_r, ti_c = tvals[i], tvals[j]
            if ti_r > 0 and ti_r < ti_c:
                row.append("np")
            elif ti_c > 0 and ti_r > ti_c:
                row.append("pn")
            else:
                row.append("pp")
        regimes.append(row)
    return regimes, True


def _build_program(regimes, qn_blocks, kn_blocks, dev_rot_kn):
    import concourse.bass as bass  # noqa: F401
    import concourse.tile as tile
    from concourse import bacc, mybir

    f16 = mybir.dt.float16
    f32 = mybir.dt.float32

    nqn = max(1, len(qn_blocks))
    nkn = max(1, len(kn_blocks))
    qn_pos = {b: idx for idx, b in enumerate(qn_blocks)}
    kn_pos = {b: idx for idx, b in enumerate(kn_blocks)}
    TK = nkn * BLK

    nc = bacc.Bacc(None, target_bir_lowering=False)
    qp_d = nc.dram_tensor("qp", [H, 2, HALF, S], f16, kind="ExternalInput")
    qn_d = nc.dram_tensor("qn", [H, 2, HALF, nqn * BLK], f16, kind="ExternalInput")
    kp_d = nc.dram_tensor("kp", [H, 2, HALF, S], f16, kind="ExternalInput")
    if dev_rot_kn:
        # [cos2|sin2|cos2] table for the kn token run; kn is derived on-device
        # from kp via the exact identity rope_-(k) = R(-2θ)·rope_+(k). The
        # overlapping views [0:2T]=[c2|s2] and [T:3T]=[s2|c2] give both
        # operand orders for the fused [pe|po] elementwise products.
        kt_d = nc.dram_tensor("kt", [HALF, 3 * TK], f16, kind="ExternalInput")
    else:
        kn_d = nc.dram_tensor("kn", [H, 2, HALF, TK], f16, kind="ExternalInput")
    out_d = nc.dram_tensor("out", [S, S, H], f32, kind="ExternalOutput")

    with tile.TileContext(nc) as tc:
        with (
            tc.tile_pool(name="inp", bufs=1) as inp,
            tc.tile_pool(name="psum", bufs=8, space="PSUM") as pp,
            tc.tile_pool(name="stage", bufs=3) as stp,
            tc.tile_pool(name="rtmp", bufs=4) as rtmp,
        ):
            # Load all inputs. Tiles are (128 partitions = pair index,
            # H*2*T tokens) fp16.
            qp_t = inp.tile([HALF, H * 2 * S], f16, tag="qp")
            qn_t = inp.tile([HALF, H * 2 * nqn * BLK], f16, tag="qn")
            kp_t = inp.tile([HALF, H * 2 * S], f16, tag="kp")
            kn_t = inp.tile([HALF, H * 2 * TK], f16, tag="kn")
            # All input DMAs go on the SP HWDGE ring ahead of the output
            # stream: small rotation table first, then qp/kp split by d-chunk
            # half (row-0 c=0 matmuls start after the first two big DMAs),
            # then qn. This packs the DMA timeline with zero idle.
            if dev_rot_kn:
                kt_t = inp.tile([HALF, 3 * TK], f16, tag="kt")
                nc.sync.dma_start(kt_t[:], kt_d[:])
            qp_v = qp_t[:].rearrange("p (h c t) -> p h c t", h=H, c=2, t=S)
            kp_v = kp_t[:].rearrange("p (h c t) -> p h c t", h=H, c=2, t=S)
            qp_dv = qp_d[:].rearrange("h c p t -> p h c t")
            kp_dv = kp_d[:].rearrange("h c p t -> p h c t")
            nc.sync.dma_start(qp_v[:, :, 0], qp_dv[:, :, 0])
            nc.sync.dma_start(kp_v[:, :, 0], kp_dv[:, :, 0])
            # rows 0-1's second-chunk lhsT (qp blocks 0-1, c=1) lands before
            # the big kp_c1 transfer so the first output half-rows are ready
            # the moment the input stream drains. Two blocks, not one: 256
            # tokens make 512B DMA descriptor rows (full rate; a single
            # 128-token block would be 256B rows at half rate).
            nc.sync.dma_start(
                qp_v[:, :, 1, 0 : 2 * BLK], qp_dv[:, :, 1, 0 : 2 * BLK]
            )
            nc.sync.dma_start(kp_v[:, :, 1], kp_dv[:, :, 1])
            nc.sync.dma_start(
                qp_v[:, :, 1, 2 * BLK : S], qp_dv[:, :, 1, 2 * BLK : S]
            )
            for c in range(2):
                tlen = nqn * BLK
                nc.sync.dma_start(
                    qn_t[:].rearrange("p (h c t) -> p h c t", h=H, c=2, t=tlen)[
                        :, :, c
                    ],
                    qn_d[:].rearrange("h c p t -> p h c t")[:, :, c],
                )
            if not dev_rot_kn:
                nc.sync.dma_start(
                    kn_t[:].rearrange("p (h c t) -> p h c t", h=H, c=2, t=TK),
                    kn_d[:].rearrange("h c p t -> p h c t"),
                )
            def emit_rotation():
                # kn = R(-2θ) kp on the kn token run, per head:
                #   kn_e = pe*cos2 + po*sin2 ; kn_o = po*cos2 - pe*sin2
                # Fused as X=[pe|po]*[c2|s2], Y=[pe|po]*[s2|c2]:
                #   kn_e = X.lo + X.hi ; kn_o = Y.hi - Y.lo
                o0 = kn_blocks[0] * BLK
                tabA = kt_t[:, 0 : 2 * TK].rearrange("p (c t) -> p c t", c=2)
                tabB = kt_t[:, TK : 3 * TK].rearrange("p (c t) -> p c t", c=2)
                for h in range(H):
                    pepo = (
                        kp_t[:]
                        .rearrange("p (h c t) -> p h c t", h=H, c=2, t=S)[
                            :, h, :, o0 : o0 + TK
                        ]
                    )  # (p, 2, TK): [pe | po]
                    tx = rtmp.tile([HALF, 2 * TK], f16, tag="tx")
                    ty = rtmp.tile([HALF, 2 * TK], f16, tag="ty")
                    tx_v = tx[:].rearrange("p (c t) -> p c t", c=2)
                    ty_v = ty[:].rearrange("p (c t) -> p c t", c=2)
                    nc.vector.tensor_mul(tx_v, pepo, tabA)
                    nc.vector.tensor_mul(ty_v, pepo, tabB)
                    nc.vector.tensor_add(
                        kn_t[:, (h * 2 + 0) * TK :][:, :TK],
                        tx[:, 0:TK],
                        tx[:, TK : 2 * TK],
                    )
                    nc.vector.tensor_sub(
                        kn_t[:, (h * 2 + 1) * TK :][:, :TK],
                        ty[:, TK : 2 * TK],
                        ty[:, 0:TK],
                    )

            def lhs_slice(variant, h, c, blk):
                if variant == "p":
                    return qp_t[:, (h * 2 + c) * S + blk * BLK :][:, :BLK]
                return qn_t[:, (h * 2 + c) * (nqn * BLK) + qn_pos[blk] * BLK :][:, :BLK]

            def rhs_slice(variant, h, c, blk):
                if variant == "p":
                    return kp_t[:, (h * 2 + c) * S + blk * BLK :][:, :BLK]
                return kn_t[:, (h * 2 + c) * (nkn * BLK) + kn_pos[blk] * BLK :][:, :BLK]

            copy_parity = 0
            for i in range(NB):
                stage = stp.tile([BLK, S * H], f32, tag="stage")
                # One PSUM bank per (i, j) holds all 4 heads [h0|h1|h2|h3].
                # Only the first matmul into the bank uses start=True (the
                # bank-wide pending-zero clear); every element is written
                # exactly once per chunk, so per-element has_written handles
                # the rest. Emit all c=0 matmuls of the row before the c=1
                # matmuls so the PE FIFO isn't head-of-line blocked waiting
                # for the second-chunk input DMA.
                banks = {}
                for j in range(NB):
                    reg = regimes[i][j]
                    qv = "n" if reg == "np" else "p"
                    kv = "n" if reg == "pn" else "p"
                    bank = pp.tile([BLK, BLK * H], f32, tag="bank")
                    banks[j] = bank
                    for h in range(H):
                        nc.tensor.matmul(
                            bank[:, h * BLK : (h + 1) * BLK],
                            lhs_slice(qv, h, 0, i),
                            rhs_slice(kv, h, 0, j),
                            start=(h == 0),
                            stop=False,
                        )
                for j in range(NB):
                    reg = regimes[i][j]
                    qv = "n" if reg == "np" else "p"
                    kv = "n" if reg == "pn" else "p"
                    bank = banks[j]
                    for h in range(H):
                        nc.tensor.matmul(
                            bank[:, h * BLK : (h + 1) * BLK],
                            lhs_slice(qv, h, 1, i),
                            rhs_slice(kv, h, 1, j),
                            start=False,
                            stop=(h == H - 1),
                        )
                    # one head-interleaving evacuation copy per bank:
                    # bank (p, (h n)) -> stage (p, (n h)) at block j
                    dst_blk = stage[:, j * (BLK * H) : (j + 1) * (BLK * H)]
                    dst_blk = dst_blk.rearrange("p (n h) -> p h n", h=H)
                    src_blk = bank[:].rearrange("p (h n) -> p h n", n=BLK)
                    # While DVE is busy with the kn rotation (early rows),
                    # route evacuation copies to ACT — except row 0's j=1,
                    # which DVE handles ahead of the rotation in its FIFO so
                    # the first output half-row is ready when the input
                    # stream drains.
                    if dev_rot_kn and i < 3:
                        use_vector = i == 0 and j == 1
                    else:
                        use_vector = copy_parity == 0
                        copy_parity ^= 1
                    if use_vector:
                        nc.vector.tensor_copy(dst_blk, src_blk)
                    else:
                        nc.scalar.copy(dst_blk, src_blk)
                # Two half-row output DMAs so the stream isn't gated on the
                # whole row's evacuation (row 0's first half is the critical
                # first transfer after the input stream drains).
                HW2 = NB // 2 * BLK * H
                nc.sync.dma_start(
                    out_d[i * BLK : (i + 1) * BLK, 0 : S // 2].rearrange(
                        "p n h -> p (n h)"
                    ),
                    stage[:, 0:HW2],
                )
                nc.sync.dma_start(
                    out_d[i * BLK : (i + 1) * BLK, S // 2 : S].rearrange(
                        "p n h -> p (n h)"
                    ),
                    stage[:, HW2 : 2 * HW2],
                )
                # kn rotation emitted after row 0 so its DVE ops queue behind
                # row 0's j=1 evacuation copy, not ahead of it.
                if dev_rot_kn and i == 0:
                    emit_rotation()
    nc.finalize()
    return nc


def _reference_fallback(qw, kw, token_index, thread_id):
    """Pure numpy fallback for unexpected block structure."""
    rots = _host_rotations(qw, kw, token_index)
    (qp_u, qp_v), (qn_u, qn_v), (kp_u, kp_v), (kn_u, kn_v) = rots

    def interleave(u, v):
        x = np.empty(u.shape[:-1] + (D,), dtype=np.float32)
        x[..., 0::2] = u
        x[..., 1::2] = v
        return x

    q_p = interleave(qp_u, qp_v)
    q_n = interleave(qn_u, qn_v)
    k_p = interleave(kp_u, kp_v)
    k_n = interleave(kn_u, kn_v)
    s_pp = np.einsum("bmhd,bnhd->bmnh", q_p, k_p)
    s_np = np.einsum("bmhd,bnhd->bmnh", q_n, k_p)
    s_pn = np.einsum("bmhd,bnhd->bmnh", q_p, k_n)
    ti_r = thread_id[:, None]
    ti_c = thread_id[None, :]
    sx = ((ti_r > 0) & (ti_r < ti_c))[None, :, :, None]
    sy = ((ti_c > 0) & (ti_r > ti_c))[None, :, :, None]
    return np.where(sx, s_np, np.where(sy, s_pn, s_pp)).astype(np.float32)


def kernel(qw, kw, token_index, thread_id):
    qw = np.asarray(qw, dtype=np.float32)
    kw = np.asarray(kw, dtype=np.float32)
    token_index = np.asarray(token_index)
    thread_id = np.asarray(thread_id)

    regimes, ok = _regime_map(thread_id)
    if (
        not ok
        or qw.shape != (B, S, H, D)
        or kw.shape != (B, S, H, D)
        or token_index.shape != (S,)
    ):
        return _reference_fallback(qw, kw, token_index, thread_id)

    qn_blocks = sorted({i for i in range(NB) if any(regimes[i][j] == "np" for j in range(NB))})
    kn_blocks = sorted({j for j in range(NB) if any(regimes[i][j] == "pn" for i in range(NB))})
    if not qn_blocks:
        qn_blocks = [0]
    if not kn_blocks:
        kn_blocks = [0]

    rots = _host_rotations(qw, kw, token_index)
    (qp_u, qp_v), (qn_u, qn_v), (kp_u, kp_v), (kn_u, kn_v) = rots
    all_blocks = list(range(NB))
    qp_a = _to_device_layout(qp_u, qp_v, all_blocks)  # (B,H,2,HALF,S)
    qn_a = _to_device_layout(qn_u, qn_v, qn_blocks)
    kp_a = _to_device_layout(kp_u, kp_v, all_blocks)

    # kn is derived on-device from kp when its blocks form one contiguous run
    # (saves its DMA); otherwise ship it like the others.
    dev_rot_kn = kn_blocks == list(range(kn_blocks[0], kn_blocks[0] + len(kn_blocks)))
    if dev_rot_kn:
        cols = np.concatenate(
            [np.arange(b * BLK, (b + 1) * BLK) for b in kn_blocks]
        )
        inv_freq = np.power(
            np.float32(ROPE_BASE),
            (np.arange(HALF, dtype=np.float32) * np.float32(-2.0 / D)),
        )
        theta = token_index[cols].astype(np.float32)[:, None] * inv_freq[None, :]
        c2 = np.cos(2.0 * theta).T  # (HALF, TK)
        s2 = np.sin(2.0 * theta).T
        kt_a = np.ascontiguousarray(
            np.concatenate([c2, s2, c2], axis=1).astype(np.float16)
        )
    else:
        kn_a = _to_device_layout(kn_u, kn_v, kn_blocks)

    key = (
        tuple(tuple(r) for r in regimes),
        tuple(qn_blocks),
        tuple(kn_blocks),
        dev_rot_kn,
    )
    if key not in _prog_cache:
        _prog_cache[key] = _build_program(regimes, qn_blocks, kn_blocks, dev_rot_kn)
    nc = _prog_cache[key]

    from concourse.bass_utils import run_bass_kernel_spmd

    in_maps = [
        {"qp": qp_a[b], "qn": qn_a[b], "kp": kp_a[b]} for b in range(B)
    ]
    for b in range(B):
        if dev_rot_kn:
            in_maps[b]["kt"] = kt_a
        else:
            in_maps[b]["kn"] = kn_a[b]
    trace = bool(int(os.environ.get("KERNEL_TRACE", "0")))
    res = None
    for attempt in range(3):
        try:
            res = run_bass_kernel_spmd(
                nc,
                in_maps,
                core_ids=list(range(N_CORES)),
                trace=trace,
            )
            break
        except Exception:
            # transient NRT/device blips (e.g. NRT_EXEC_UNIT_UNRECOVERABLE)
            # have been observed on otherwise-correct programs; retry.
            if attempt == 2:
                raise
    if res.exec_time_ns is not None:
        print(f"HW exec time: {res.exec_time_ns} ns")
    if res.instructions_and_trace is not None:
        print(f"trace: {res.instructions_and_trace[1]}")

    out = np.stack([res.results[b]["out"] for b in range(B)], axis=0)
    return out.astype(np.float32)



# revision 2
# speedup vs baseline: 1.2341x; 1.2341x over previous
"""Trainium2 Bass kernel for nn_BertWordPair (ragged RoPE pair scores).

Strategy
--------
Inputs: qw, kw (B=8, S=768, H=4, D=256) fp32; token_index, thread_id (S,) int32.
Output: (B, S, S, H) fp32 where each (row-block, col-block) pair of the 6x128
thread-block grid uses one of three RoPE sign regimes:
    pp: rope(q,+pos) . rope(k,+pos)
    np: rope(q,-pos) . rope(k,+pos)   (0 < ti_r < ti_c)
    pn: rope(q,+pos) . rope(k,-pos)   (ti_c > 0, ti_r > ti_c)

Host side precomputes the +rotated variants qp, kp in a de-interleaved
(pair-index, token) layout, casts to fp16, and shards batch across the 8
cores (1 dialogue per core). The -rotated variants qn, kn are derived
on-device from qp, kp by a DVE fp16 rotation (x- = R(-2θ)x+) using one
small per-block-pattern [cos2|sin2|cos2] table broadcast across heads,
saving their DMA entirely. Device work: matmuls (one 128x128x256
contraction per output block/head, fp16 in, fp32 PSUM, 4 heads packed
per PSUM bank), one head-interleaving fp32->fp16 PSUM->SBUF copy per
bank (spread across ACT/Pool/DVE), and half-row fp16 output DMAs (the
host upcasts to fp32). The DMA ring carries ~7.9MB per core
(qp+kp+table in, fp16 scores out) and is ordered to stay gapless; the
DVE rotation ladder is interleaved (qn_b, kn_b alternating by output
deadline) so each derived block lands before the row that needs it.
"""

import os

import numpy as np

ROPE_BASE = 10000.0
B, S, H, D = 8, 768, 4, 256
HALF = D // 2  # 128
BLK = 128
NB = S // BLK  # 6
N_CORES = 8

_prog_cache = {}


def _host_rotations(qw, kw, token_index):
    """Return u/v (even/odd) rotated variants, fp32.

    Shapes: (B, S, H, HALF) each for (qp_u, qp_v, qn_u, qn_v, kp_u, kp_v,
    kn_u, kn_v)."""
    inv_freq = np.power(
        np.float32(ROPE_BASE),
        (np.arange(HALF, dtype=np.float32) * np.float32(-2.0 / D)),
    )  # (HALF,)
    pos = token_index.astype(np.float32)  # (S,)
    theta = pos[:, None] * inv_freq[None, :]  # (S, HALF)
    cos = np.cos(theta)[None, :, None, :]  # (1,S,1,HALF)
    sin = np.sin(theta)[None, :, None, :]

    out = []
    for x in (qw, kw):
        u = x[..., 0::2]  # (B,S,H,HALF)
        v = x[..., 1::2]
        uc = u * cos
        vs = v * sin
        vc = v * cos
        us = u * sin
        # positive rotation
        out.append((uc - vs, vc + us))
        # negative rotation (sin -> -sin)
        out.append((uc + vs, vc - us))
    return out  # [(qp_u,qp_v),(qn_u,qn_v),(kp_u,kp_v),(kn_u,kn_v)]


def _to_device_layout(u, v):
    """(B,S,H,HALF) u/v -> (B, H, 2, HALF, S) fp16."""
    arr = np.stack([u, v], axis=2)  # (B,S,2,H,HALF)
    arr = np.transpose(arr, (0, 3, 2, 4, 1))  # (B,H,2,HALF,S)
    return np.ascontiguousarray(arr.astype(np.float16))


def _plan(token_index, thread_id):
    """Build the static execution plan, or None if the structure doesn't
    match the fast path (aligned 128-blocks, shared per-block position
    pattern)."""
    tid = np.asarray(thread_id)
    tok = np.asarray(token_index)
    if tid.shape != (S,) or tok.shape != (S,):
        return None
    blocks = tid.reshape(NB, BLK)
    if not np.all(blocks == blocks[:, :1]):
        return None  # thread blocks not aligned to the 128 grid
    # all blocks must share one position pattern (the rotation table is
    # built once and reused for every derived block)
    pat = tok.reshape(NB, BLK)
    if not np.all(pat == pat[:1]):
        return None
    tvals = blocks[:, 0]
    regimes = []
    for i in range(NB):
        row = []
        for j in range(NB):
            ti_r, ti_c = tvals[i], tvals[j]
            if ti_r > 0 and ti_r < ti_c:
                row.append("np")
            elif ti_c > 0 and ti_r > ti_c:
                row.append("pn")
            else:
                row.append("pp")
        regimes.append(row)

    # rotation ladder: one derived 128-block per DVE step, ordered by the
    # first output half-row slot that consumes it. Slot of (row i, half x)
    # = 2*i + x; half 0 covers banks j=0..2, half 1 j=3..5.
    need = {}
    for i in range(NB):
        for j in range(NB):
            reg = regimes[i][j]
            if reg == "pp":
                continue
            slot = 2 * i + (0 if j < NB // 2 else 1)
            key = ("q", i) if reg == "np" else ("k", j)
            need[key] = min(need.get(key, 1 << 30), slot)
    ladder = sorted(need, key=lambda k: (need[k], k))
    return {
        "regimes": tuple(tuple(r) for r in regimes),
        "ladder": tuple(ladder),
    }


def _build_program(plan):
    import concourse.bass as bass  # noqa: F401
    import concourse.tile as tile
    from concourse import bacc, mybir

    f16 = mybir.dt.float16
    f32 = mybir.dt.float32

    regimes = plan["regimes"]
    ladder = plan["ladder"]

    nc = bacc.Bacc(None, target_bir_lowering=False)
    qp_d = nc.dram_tensor("qp", [H, 2, HALF, S], f16, kind="ExternalInput")
    kp_d = nc.dram_tensor("kp", [H, 2, HALF, S], f16, kind="ExternalInput")
    # [cos2|sin2|cos2] over one 128-token block pattern; the overlapping
    # views [0:2T]=[c2|s2] and [T:3T]=[s2|c2] give both operand orders for
    # the fused [even|odd] elementwise products of x- = R(-2θ)x+.
    kt_d = nc.dram_tensor("kt", [HALF, 3 * BLK], f16, kind="ExternalInput")
    out_d = nc.dram_tensor("out", [S, S, H], f16, kind="ExternalOutput")

    with tile.TileContext(nc) as tc:
        with (
            tc.tile_pool(name="inp", bufs=1) as inp,
            tc.tile_pool(name="psum", bufs=8, space="PSUM") as pp,
            tc.tile_pool(name="stage", bufs=3) as stp,
            tc.tile_pool(name="rtmp", bufs=4) as rtmp,
        ):
            # Input tiles: (128 partitions = pair index, H*2*S tokens) fp16.
            qp_t = inp.tile([HALF, H * 2 * S], f16, tag="qp")
            kp_t = inp.tile([HALF, H * 2 * S], f16, tag="kp")
            qn_t = inp.tile([HALF, H * 2 * S], f16, tag="qn")
            kn_t = inp.tile([HALF, H * 2 * S], f16, tag="kn")
            kt_t = inp.tile([HALF, 3 * BLK], f16, tag="kt")

            qp_v = qp_t[:].rearrange("p (h c t) -> p h c t", h=H, c=2, t=S)
            kp_v = kp_t[:].rearrange("p (h c t) -> p h c t", h=H, c=2, t=S)
            qp_dv = qp_d[:].rearrange("h c p t -> p h c t")
            kp_dv = kp_d[:].rearrange("h c p t -> p h c t")

            # Input DMA order (single SP HWDGE ring, ahead of the output
            # stream): table, q/k first chunks, kp second chunk (row 0 is
            # all-pp and needs every kp block), then qp second chunk with
            # blocks 0-1 first (row 0/1 lhsT + the first qn ladder steps).
            nc.sync.dma_start(kt_t[:], kt_d[:])
            nc.sync.dma_start(qp_v[:, :, 0], qp_dv[:, :, 0])
            nc.sync.dma_start(kp_v[:, :, 0], kp_dv[:, :, 0])
            nc.sync.dma_start(kp_v[:, :, 1], kp_dv[:, :, 1])
            nc.sync.dma_start(
                qp_v[:, :, 1, 0 : 2 * BLK], qp_dv[:, :, 1, 0 : 2 * BLK]
            )
            nc.sync.dma_start(
                qp_v[:, :, 1, 2 * BLK : S], qp_dv[:, :, 1, 2 * BLK : S]
            )

            # x- = R(-2θ) x+ per derived 128-block, all 4 heads fused into
            # one op via a stride-0 head broadcast of the table:
            #   x-_e = e*cos2 + o*sin2 ; x-_o = o*cos2 - e*sin2
            # As X=[e|o]*[c2|s2], Y=[e|o]*[s2|c2]:
            #   x-_e = X.e + X.o ; x-_o = Y.o - Y.e
            tabA = (
                kt_t[:, 0 : 2 * BLK]
                .rearrange("p (c t) -> p c t", c=2)
                .unsqueeze(1)
                .broadcast_to([HALF, H, 2, BLK])
            )
            tabB = (
                kt_t[:, BLK : 3 * BLK]
                .rearrange("p (c t) -> p c t", c=2)
                .unsqueeze(1)
                .broadcast_to([HALF, H, 2, BLK])
            )

            def emit_rotation(variant, b):
                src_v = qp_v if variant == "q" else kp_v
                dst_t = qn_t if variant == "q" else kn_t
                pepo = src_v[:, :, :, b * BLK : (b + 1) * BLK]  # (p,h,c,t)
                dst_v = dst_t[:].rearrange(
                    "p (h c t) -> p h c t", h=H, c=2, t=S
                )[:, :, :, b * BLK : (b + 1) * BLK]
                tx = rtmp.tile([HALF, H * 2 * BLK], f16, tag="tx")
                ty = rtmp.tile([HALF, H * 2 * BLK], f16, tag="ty")
                tx_v = tx[:].rearrange("p (h c t) -> p h c t", h=H, c=2, t=BLK)
                ty_v = ty[:].rearrange("p (h c t) -> p h c t", h=H, c=2, t=BLK)
                nc.vector.tensor_mul(tx_v, pepo, tabA)
                nc.vector.tensor_mul(ty_v, pepo, tabB)
                nc.vector.tensor_add(
                    dst_v[:, :, 0], tx_v[:, :, 0], tx_v[:, :, 1]
                )
                nc.vector.tensor_sub(
                    dst_v[:, :, 1], ty_v[:, :, 1], ty_v[:, :, 0]
                )

            def src_slice(variant, tile_t, h, c, blk):
                return tile_t[:, (h * 2 + c) * S + blk * BLK :][:, :BLK]

            def lhs_slice(reg, h, c, blk):
                t = qn_t if reg == "np" else qp_t
                return src_slice("q", t, h, c, blk)

            def rhs_slice(reg, h, c, blk):
                t = kn_t if reg == "pn" else kp_t
                return src_slice("k", t, h, c, blk)

            # Evacuation engine per (row, bank): ACT carries the most, Pool
            # a steady share, DVE only where the rotation ladder has slack
            # (row 0 while waiting for kp_c1, and the last rows).
            def evac_engine(i, j):
                plan_rows = {
                    0: "APVAPA",
                    1: "APAPAP",
                    2: "PAAPAP",
                    3: "APAPAP",
                    4: "PAVAPV",
                    5: "AVPVAV",
                }
                return plan_rows[i][j]

            ladder_iter = iter(ladder)
            emitted_rot = 0

            def emit_next_rotations(n):
                nonlocal emitted_rot
                for _ in range(n):
                    step = next(ladder_iter, None)
                    if step is None:
                        return
                    emit_rotation(*step)
                    emitted_rot += 1

            # The whole ladder is emitted up front on DVE's queue (after
            # row 0's j=2 evacuation copy slots in below via program order
            # we instead emit the first evac before... handled inline).
            for i in range(NB):
                stage = stp.tile([BLK, S * H], f16, tag="stage")
                # One PSUM bank per (i, j) holds all 4 heads [h0|h1|h2|h3].
                # Emit all c=0 matmuls of the row before the c=1 matmuls so
                # the PE FIFO isn't head-of-line blocked waiting for the
                # second-chunk input DMA. Within a chunk, pp banks first
                # (no rotation dependency).
                banks = {}
                order = sorted(
                    range(NB), key=lambda j: (regimes[i][j] != "pp", j)
                )
                for c in range(2):
                    for j in order:
                        reg = regimes[i][j]
                        if c == 0:
                            bank = pp.tile([BLK, BLK * H], f32, tag="bank")
                            banks[j] = bank
                        bank = banks[j]
                        for h in range(H):
                            nc.tensor.matmul(
                                bank[:, h * BLK : (h + 1) * BLK],
                                lhs_slice(reg, h, c, i),
                                rhs_slice(reg, h, c, j),
                                start=(c == 0 and h == 0),
                                stop=(c == 1 and h == H - 1),
                            )
                for j in range(NB):
                    bank = banks[j]
                    # one head-interleaving evacuation copy per bank:
                    # bank (p, (h n)) -> stage (p, (n h)) at block j
                    dst_blk = stage[:, j * (BLK * H) : (j + 1) * (BLK * H)]
                    dst_blk = dst_blk.rearrange("p (n h) -> p h n", h=H)
                    src_blk = bank[:].rearrange("p (h n) -> p h n", n=BLK)
                    eng = evac_engine(i, j)
                    if eng == "V":
                        nc.vector.tensor_copy(dst_blk, src_blk)
                    elif eng == "P":
                        nc.gpsimd.tensor_copy(dst_blk, src_blk)
                    else:
                        nc.scalar.copy(dst_blk, src_blk)
                # Two half-row output DMAs so the stream isn't gated on the
                # whole row's evacuation.
                HW2 = NB // 2 * BLK * H
                nc.sync.dma_start(
                    out_d[i * BLK : (i + 1) * BLK, 0 : S // 2].rearrange(
                        "p n h -> p (n h)"
                    ),
                    stage[:, 0:HW2],
                )
                nc.sync.dma_start(
                    out_d[i * BLK : (i + 1) * BLK, S // 2 : S].rearrange(
                        "p n h -> p (n h)"
                    ),
                    stage[:, HW2 : 2 * HW2],
                )
                # DVE rotation ladder: the full ladder queues behind row 0's
                # j-slot evac copy; emit it after row 0's banks so program
                # order puts that copy first on DVE.
                if i == 0:
                    emit_next_rotations(len(ladder))
    nc.finalize()
    return nc


def _reference_fallback(qw, kw, token_index, thread_id):
    """Pure numpy fallback for unexpected block structure."""
    rots = _host_rotations(qw, kw, token_index)
    (qp_u, qp_v), (qn_u, qn_v), (kp_u, kp_v), (kn_u, kn_v) = rots

    def interleave(u, v):
        x = np.empty(u.shape[:-1] + (D,), dtype=np.float32)
        x[..., 0::2] = u
        x[..., 1::2] = v
        return x

    q_p = interleave(qp_u, qp_v)
    q_n = interleave(qn_u, qn_v)
    k_p = interleave(kp_u, kp_v)
    k_n = interleave(kn_u, kn_v)
    s_pp = np.einsum("bmhd,bnhd->bmnh", q_p, k_p)
    s_np = np.einsum("bmhd,bnhd->bmnh", q_n, k_p)
    s_pn = np.einsum("bmhd,bnhd->bmnh", q_p, k_n)
    ti_r = thread_id[:, None]
    ti_c = thread_id[None, :]
    sx = ((ti_r > 0) & (ti_r < ti_c))[None, :, :, None]
    sy = ((ti_c > 0) & (ti_r > ti_c))[None, :, :, None]
    return np.where(sx, s_np, np.where(sy, s_pn, s_pp)).astype(np.float32)


def _rotation_table(token_index):
    """[c2|s2|c2] fp16 table (HALF, 3*BLK) for one block's position pattern."""
    inv_freq = np.power(
        np.float32(ROPE_BASE),
        (np.arange(HALF, dtype=np.float32) * np.float32(-2.0 / D)),
    )
    theta = token_index[:BLK].astype(np.float32)[:, None] * inv_freq[None, :]
    c2 = np.cos(2.0 * theta).T  # (HALF, BLK)
    s2 = np.sin(2.0 * theta).T
    return np.ascontiguousarray(
        np.concatenate([c2, s2, c2], axis=1).astype(np.float16)
    )


def kernel(qw, kw, token_index, thread_id):
    qw = np.asarray(qw, dtype=np.float32)
    kw = np.asarray(kw, dtype=np.float32)
    token_index = np.asarray(token_index)
    thread_id = np.asarray(thread_id)

    plan = _plan(token_index, thread_id)
    if plan is None or qw.shape != (B, S, H, D) or kw.shape != (B, S, H, D):
        return _reference_fallback(qw, kw, token_index, thread_id)

    rots = _host_rotations(qw, kw, token_index)
    (qp_u, qp_v), _, (kp_u, kp_v), _ = rots
    qp_a = _to_device_layout(qp_u, qp_v)  # (B,H,2,HALF,S)
    kp_a = _to_device_layout(kp_u, kp_v)
    kt_a = _rotation_table(token_index)

    key = plan["regimes"]
    if key not in _prog_cache:
        _prog_cache[key] = _build_program(plan)
    nc = _prog_cache[key]

    from concourse.bass_utils import run_bass_kernel_spmd

    in_maps = [
        {"qp": qp_a[b], "kp": kp_a[b], "kt": kt_a} for b in range(B)
    ]
    trace = bool(int(os.environ.get("KERNEL_TRACE", "0")))
    res = None
    for attempt in range(3):
        try:
            res = run_bass_kernel_spmd(
                nc,
                in_maps,
                core_ids=list(range(N_CORES)),
                trace=trace,
            )
            break
        except Exception:
            # transient NRT/device blips (e.g. NRT_EXEC_UNIT_UNRECOVERABLE)
            # have been observed on otherwise-correct programs; retry.
            if attempt == 2:
                raise
    if res.exec_time_ns is not None:
        print(f"HW exec time: {res.exec_time_ns} ns")
    if res.instructions_and_trace is not None:
        print(f"trace: {res.instructions_and_trace[1]}")

    out = np.stack([res.results[b]["out"] for b in range(B)], axis=0)
    return out.astype(np.float32)


# revision 3
# speedup vs baseline: 1.2470x; 1.0105x over previous
"""Trainium2 Bass kernel for nn_BertWordPair (ragged RoPE pair scores).

Strategy
--------
Inputs: qw, kw (B=8, S=768, H=4, D=256) fp32; token_index, thread_id (S,) int32.
Output: (B, S, S, H) fp32 where each (row-block, col-block) pair of the 6x128
thread-block grid uses one of three RoPE sign regimes:
    pp: rope(q,+pos) . rope(k,+pos)
    np: rope(q,-pos) . rope(k,+pos)   (0 < ti_r < ti_c)
    pn: rope(q,+pos) . rope(k,-pos)   (ti_c > 0, ti_r > ti_c)

Host side precomputes the rotated variants qp, qn, kp in a de-interleaved
(pair-index, token) layout, casts to fp16, and shards batch across the 8
cores (1 dialogue per core). kn is derived on-device from kp by a DVE
fp16 rotation (kn = R(-2θ)kp) using one small per-block-pattern
[c2|s2|-s2|c2] table broadcast across heads — two fused DVE ops per
128-block — saving its DMA. Device work: matmuls (one 128x128x256
contraction per output block/head, fp16 in, fp32 PSUM, 4 heads packed
per PSUM bank), one head-interleaving fp32->fp16 PSUM->SBUF copy per
bank (spread across ACT/Pool, DVE joining once the rotation ladder
drains), and half-row fp16 output DMAs (the host upcasts to fp32).
The SP DMA ring carries ~9.0MB per core (qp+qn+kp in, fp16 scores out),
ordered so row 0's dependencies land early and the ring never stalls;
the tiny table rides the Pool SWDGE queue off the critical ring.
"""

import os

import numpy as np

ROPE_BASE = 10000.0
B, S, H, D = 8, 768, 4, 256
HALF = D // 2  # 128
BLK = 128
NB = S // BLK  # 6
N_CORES = 8

_prog_cache = {}


def _host_rotations(qw, kw, token_index):
    """Return u/v (even/odd) rotated variants, fp32.

    Shapes: (B, S, H, HALF) each for (qp_u, qp_v, qn_u, qn_v, kp_u, kp_v,
    kn_u, kn_v)."""
    inv_freq = np.power(
        np.float32(ROPE_BASE),
        (np.arange(HALF, dtype=np.float32) * np.float32(-2.0 / D)),
    )  # (HALF,)
    pos = token_index.astype(np.float32)  # (S,)
    theta = pos[:, None] * inv_freq[None, :]  # (S, HALF)
    cos = np.cos(theta)[None, :, None, :]  # (1,S,1,HALF)
    sin = np.sin(theta)[None, :, None, :]

    out = []
    for x in (qw, kw):
        u = x[..., 0::2]  # (B,S,H,HALF)
        v = x[..., 1::2]
        uc = u * cos
        vs = v * sin
        vc = v * cos
        us = u * sin
        # positive rotation
        out.append((uc - vs, vc + us))
        # negative rotation (sin -> -sin)
        out.append((uc + vs, vc - us))
    return out  # [(qp_u,qp_v),(qn_u,qn_v),(kp_u,kp_v),(kn_u,kn_v)]


def _to_device_layout(u, v, blocks):
    """(B,S,H,HALF) u/v -> (B, H, 2, HALF, T) fp16 for the given token blocks."""
    cols = np.concatenate([np.arange(b * BLK, (b + 1) * BLK) for b in blocks])
    u = u[:, cols]  # (B,T,H,HALF)
    v = v[:, cols]
    arr = np.stack([u, v], axis=2)  # (B,T,2,H,HALF)
    arr = np.transpose(arr, (0, 3, 2, 4, 1))  # (B,H,2,HALF,T)
    return np.ascontiguousarray(arr.astype(np.float16))


def _plan(token_index, thread_id):
    """Build the static execution plan, or None if the structure doesn't
    match the fast path (aligned 128-blocks, shared per-block position
    pattern)."""
    tid = np.asarray(thread_id)
    tok = np.asarray(token_index)
    if tid.shape != (S,) or tok.shape != (S,):
        return None
    blocks = tid.reshape(NB, BLK)
    if not np.all(blocks == blocks[:, :1]):
        return None  # thread blocks not aligned to the 128 grid
    # all blocks must share one position pattern (the rotation table is
    # built once and reused for every kn block)
    pat = tok.reshape(NB, BLK)
    if not np.all(pat == pat[:1]):
        return None
    tvals = blocks[:, 0]
    regimes = []
    for i in range(NB):
        row = []
        for j in range(NB):
            ti_r, ti_c = tvals[i], tvals[j]
            if ti_r > 0 and ti_r < ti_c:
                row.append("np")
            elif ti_c > 0 and ti_r > ti_c:
                row.append("pn")
            else:
                row.append("pp")
        regimes.append(row)

    qn_blocks = sorted(
        {i for i in range(NB) if any(regimes[i][j] == "np" for j in range(NB))}
    )
    kn_blocks = sorted(
        {j for j in range(NB) if any(regimes[i][j] == "pn" for i in range(NB))}
    )
    return {
        "regimes": tuple(tuple(r) for r in regimes),
        "qn_blocks": tuple(qn_blocks),
        "kn_blocks": tuple(kn_blocks),
    }


def _build_program(plan):
    import concourse.bass as bass  # noqa: F401
    import concourse.tile as tile
    from concourse import bacc, mybir

    f16 = mybir.dt.float16
    f32 = mybir.dt.float32

    regimes = plan["regimes"]
    qn_blocks = list(plan["qn_blocks"])
    kn_blocks = list(plan["kn_blocks"])
    nqn = max(1, len(qn_blocks))
    qn_pos = {b: idx for idx, b in enumerate(qn_blocks)}
    TQ = nqn * BLK

    nc = bacc.Bacc(None, target_bir_lowering=False)
    qp_d = nc.dram_tensor("qp", [H, 2, HALF, S], f16, kind="ExternalInput")
    qn_d = nc.dram_tensor("qn", [H, 2, HALF, TQ], f16, kind="ExternalInput")
    kp_d = nc.dram_tensor("kp", [H, 2, HALF, S], f16, kind="ExternalInput")
    # [c2|s2|-s2|c2] over one 128-token block pattern: the (ab, c) view with
    # both strides = BLK gives [c2|s2] at ab=0 and [-s2|c2] at ab=1, so one
    # broadcast mul + one dual-add produce kn_e = e*c2+o*s2 (ab=0 sum) and
    # kn_o = o*c2-e*s2 (ab=1 sum) for all 4 heads at once.
    kt_d = nc.dram_tensor("kt", [HALF, 4 * BLK], f16, kind="ExternalInput")
    out_d = nc.dram_tensor("out", [S, S, H], f16, kind="ExternalOutput")

    with tile.TileContext(nc) as tc:
        with (
            tc.tile_pool(name="inp", bufs=1) as inp,
            tc.tile_pool(name="psum", bufs=8, space="PSUM") as pp,
            tc.tile_pool(name="stage", bufs=3) as stp,
            tc.tile_pool(name="rtmp", bufs=2) as rtmp,
        ):
            # Input tiles: (128 partitions = pair index, H*2*T tokens) fp16.
            qp_t = inp.tile([HALF, H * 2 * S], f16, tag="qp")
            qn_t = inp.tile([HALF, H * 2 * TQ], f16, tag="qn")
            kp_t = inp.tile([HALF, H * 2 * S], f16, tag="kp")
            kn_t = inp.tile([HALF, H * 2 * S], f16, tag="kn")
            kt_t = inp.tile([HALF, 4 * BLK], f16, tag="kt")

            qp_v = qp_t[:].rearrange("p (h c t) -> p h c t", h=H, c=2, t=S)
            kp_v = kp_t[:].rearrange("p (h c t) -> p h c t", h=H, c=2, t=S)
            qn_v = qn_t[:].rearrange("p (h c t) -> p h c t", h=H, c=2, t=TQ)
            qp_dv = qp_d[:].rearrange("h c p t -> p h c t")
            kp_dv = kp_d[:].rearrange("h c p t -> p h c t")
            qn_dv = qn_d[:].rearrange("h c p t -> p h c t")

            # The tiny rotation table rides the Pool SWDGE queue so the SP
            # ring starts straight into the big transfers.
            nc.gpsimd.dma_start(kt_t[:], kt_d[:])
            # SP ring input order: q/k first chunks, row-0's lhsT second
            # chunk (blocks 0-1), all of kp's second chunk (row 0 needs every
            # kp block; the kn ladder starts here too), then qn, then the
            # remaining qp second chunks (rows 2-5 lhsT, needed one output
            # row-slot at a time).
            nc.sync.dma_start(kp_v[:, :, 0], kp_dv[:, :, 0])
            nc.sync.dma_start(qp_v[:, :, 0], qp_dv[:, :, 0])
            nc.sync.dma_start(
                qp_v[:, :, 1, 0 : 2 * BLK], qp_dv[:, :, 1, 0 : 2 * BLK]
            )
            nc.sync.dma_start(kp_v[:, :, 1], kp_dv[:, :, 1])
            nc.sync.dma_start(qn_v[:], qn_dv[:])
            nc.sync.dma_start(
                qp_v[:, :, 1, 2 * BLK : S], qp_dv[:, :, 1, 2 * BLK : S]
            )

            # kn = R(-2θ) kp per derived 128-block, all 4 heads fused via
            # stride-0 broadcasts: txy[ab,h,c,t] = kp[h,c,t] * tab[ab,c,t],
            # then kn[c'=ab] = sum over c of txy.
            tab = (
                kt_t[:]
                .rearrange("p (ab c t) -> p ab c t", ab=2, c=2)
                .unsqueeze(2)
                .broadcast_to([HALF, 2, H, 2, BLK])
            )

            def emit_rotation(b):
                pepo = (
                    kp_v[:, :, :, b * BLK : (b + 1) * BLK]
                    .unsqueeze(1)
                    .broadcast_to([HALF, 2, H, 2, BLK])
                )
                txy = rtmp.tile([HALF, 2 * H * 2 * BLK], f16, tag="txy")
                txy_v = txy[:].rearrange(
                    "p (ab h c t) -> p ab h c t", ab=2, h=H, c=2, t=BLK
                )
                nc.vector.tensor_mul(txy_v, pepo, tab)
                dst = kn_t[:].rearrange(
                    "p (h c t) -> p c h t", h=H, c=2, t=S
                )[:, :, :, b * BLK : (b + 1) * BLK]
                nc.vector.tensor_add(
                    dst, txy_v[:, :, :, 0, :], txy_v[:, :, :, 1, :]
                )

            def lhs_slice(reg, h, c, blk):
                if reg == "np":
                    return qn_t[:, (h * 2 + c) * TQ + qn_pos[blk] * BLK :][
                        :, :BLK
                    ]
                return qp_t[:, (h * 2 + c) * S + blk * BLK :][:, :BLK]

            def rhs_slice(reg, h, c, blk):
                t = kn_t if reg == "pn" else kp_t
                return t[:, (h * 2 + c) * S + blk * BLK :][:, :BLK]

            # Evacuation engine per (row, bank): ACT + Pool carry the early
            # rows; DVE joins once the kn rotation ladder has drained.
            evac_plan = {
                0: "APAPAP",
                1: "PAAPAA",
                2: "APAPAA",
                3: "APVAPA",
                4: "VAPVAP",
                5: "VAVPAV",
            }

            def emit_row(i, stage):
                banks = {}
                order = sorted(
                    range(NB), key=lambda j: (regimes[i][j] != "pp", j)
                )
                for c in range(2):
                    for j in order:
                        reg = regimes[i][j]
                        if c == 0:
                            bank = pp.tile([BLK, BLK * H], f32, tag="bank")
                            banks[j] = bank
                        bank = banks[j]
                        for h in range(H):
                            nc.tensor.matmul(
                                bank[:, h * BLK : (h + 1) * BLK],
                                lhs_slice(reg, h, c, i),
                                rhs_slice(reg, h, c, j),
                                start=(c == 0 and h == 0),
                                stop=(c == 1 and h == H - 1),
                            )
                for j in range(NB):
                    bank = banks[j]
                    # one head-interleaving evacuation copy per bank:
                    # bank (p, (h n)) -> stage (p, (n h)) at block j
                    dst_blk = stage[:, j * (BLK * H) : (j + 1) * (BLK * H)]
                    dst_blk = dst_blk.rearrange("p (n h) -> p h n", h=H)
                    src_blk = bank[:].rearrange("p (h n) -> p h n", n=BLK)
                    eng = evac_plan[i][j]
                    if eng == "V":
                        nc.vector.tensor_copy(dst_blk, src_blk)
                    elif eng == "P":
                        nc.gpsimd.tensor_copy(dst_blk, src_blk)
                    else:
                        nc.scalar.copy(dst_blk, src_blk)
                # Two half-row output DMAs so the stream isn't gated on the
                # whole row's evacuation.
                HW2 = NB // 2 * BLK * H
                nc.sync.dma_start(
                    out_d[i * BLK : (i + 1) * BLK, 0 : S // 2].rearrange(
                        "p n h -> p (n h)"
                    ),
                    stage[:, 0:HW2],
                )
                nc.sync.dma_start(
                    out_d[i * BLK : (i + 1) * BLK, S // 2 : S].rearrange(
                        "p n h -> p (n h)"
                    ),
                    stage[:, HW2 : 2 * HW2],
                )

            for i in range(NB):
                stage = stp.tile([BLK, S * H], f16, tag="stage")
                emit_row(i, stage)
                # kn ladder: emitted after row 0 so DVE's queue holds the
                # rotations ahead of any row 1+ work it may pick up; each
                # block's ops only depend on kp_c1 + the table.
                if i == 0:
                    for b in kn_blocks:
                        emit_rotation(b)
    nc.finalize()
    return nc


def _reference_fallback(qw, kw, token_index, thread_id):
    """Pure numpy fallback for unexpected block structure."""
    rots = _host_rotations(qw, kw, token_index)
    (qp_u, qp_v), (qn_u, qn_v), (kp_u, kp_v), (kn_u, kn_v) = rots

    def interleave(u, v):
        x = np.empty(u.shape[:-1] + (D,), dtype=np.float32)
        x[..., 0::2] = u
        x[..., 1::2] = v
        return x

    q_p = interleave(qp_u, qp_v)
    q_n = interleave(qn_u, qn_v)
    k_p = interleave(kp_u, kp_v)
    k_n = interleave(kn_u, kn_v)
    s_pp = np.einsum("bmhd,bnhd->bmnh", q_p, k_p)
    s_np = np.einsum("bmhd,bnhd->bmnh", q_n, k_p)
    s_pn = np.einsum("bmhd,bnhd->bmnh", q_p, k_n)
    ti_r = thread_id[:, None]
    ti_c = thread_id[None, :]
    sx = ((ti_r > 0) & (ti_r < ti_c))[None, :, :, None]
    sy = ((ti_c > 0) & (ti_r > ti_c))[None, :, :, None]
    return np.where(sx, s_np, np.where(sy, s_pn, s_pp)).astype(np.float32)


def _rotation_table(token_index):
    """[c2|s2|-s2|c2] fp16 table (HALF, 4*BLK) for one block's pattern."""
    inv_freq = np.power(
        np.float32(ROPE_BASE),
        (np.arange(HALF, dtype=np.float32) * np.float32(-2.0 / D)),
    )
    theta = token_index[:BLK].astype(np.float32)[:, None] * inv_freq[None, :]
    c2 = np.cos(2.0 * theta).T  # (HALF, BLK)
    s2 = np.sin(2.0 * theta).T
    return np.ascontiguousarray(
        np.concatenate([c2, s2, -s2, c2], axis=1).astype(np.float16)
    )


def kernel(qw, kw, token_index, thread_id):
    qw = np.asarray(qw, dtype=np.float32)
    kw = np.asarray(kw, dtype=np.float32)
    token_index = np.asarray(token_index)
    thread_id = np.asarray(thread_id)

    plan = _plan(token_index, thread_id)
    if plan is None or qw.shape != (B, S, H, D) or kw.shape != (B, S, H, D):
        return _reference_fallback(qw, kw, token_index, thread_id)

    rots = _host_rotations(qw, kw, token_index)
    (qp_u, qp_v), (qn_u, qn_v), (kp_u, kp_v), _ = rots
    all_blocks = list(range(NB))
    qn_blocks = list(plan["qn_blocks"]) or [0]
    qp_a = _to_device_layout(qp_u, qp_v, all_blocks)  # (B,H,2,HALF,S)
    qn_a = _to_device_layout(qn_u, qn_v, qn_blocks)
    kp_a = _to_device_layout(kp_u, kp_v, all_blocks)
    kt_a = _rotation_table(token_index)

    key = plan["regimes"]
    if key not in _prog_cache:
        _prog_cache[key] = _build_program(plan)
    nc = _prog_cache[key]

    from concourse.bass_utils import run_bass_kernel_spmd

    in_maps = [
        {"qp": qp_a[b], "qn": qn_a[b], "kp": kp_a[b], "kt": kt_a}
        for b in range(B)
    ]
    trace = bool(int(os.environ.get("KERNEL_TRACE", "0")))
    res = None
    for attempt in range(3):
        try:
            res = run_bass_kernel_spmd(
                nc,
                in_maps,
                core_ids=list(range(N_CORES)),
                trace=trace,
            )
            break
        except Exception:
            # transient NRT/device blips (e.g. NRT_EXEC_UNIT_UNRECOVERABLE)
            # have been observed on otherwise-correct programs; retry.
            if attempt == 2:
                raise
    if res.exec_time_ns is not None:
        print(f"HW exec time: {res.exec_time_ns} ns")
    if res.instructions_and_trace is not None:
        print(f"trace: {res.instructions_and_trace[1]}")

    out = np.stack([res.results[b]["out"] for b in range(B)], axis=0)
    return out.astype(np.float32)
